# revision 61
# baseline (speedup 1.0000x reference)
"""ReActNet BasicBlock (binary conv 3x3 256->256 + sync-BN + ReLU) on 8 trn2 cores.

Math: forward(x) = relu(BN(conv(sign(x), scale*sign(w)))), BN over full batch.
Since sign(x) in {-1,0,1} and sign(w) in {-1,1}, the conv reduces to an exact
integer-valued convolution s = conv(sign(x), sign(w)) computable in bf16
matmuls with fp32 PSUM accumulation (products are +-1, exactly representable).
y = scale[o] * s, so BN+affine folds into out = relu(a[o]*s + b[o]) with
  a = gamma*scale / sqrt(scale^2*var_s + eps),  b = beta - a*mean_s
where mean_s/var_s are batch stats of s (all-reduced across the 8 cores).

Sharding: data-parallel over batch (32 -> 4 images/core); weights + BN params
replicated; one 2KB AllReduce for the sync-BN statistics.
"""
import numpy as np
import ml_dtypes

import concourse.bacc as bacc
from concourse import mybir, tile
from concourse.bass_utils import run_bass_kernel_spmd

N_CORES = 8
B = 4               # images per core
C = 256             # channels
H = W = 56
PW = 58             # padded row width
IMG = PW * PW       # padded image size (58x58)
SLK = 64            # zero slack before each padded image
XAF = 3504          # SLK + IMG + end slack, multiple of 16 (DoubleRow AP step)
# split-image activation tiles: top holds padded rows 0..33 (input rows
# 0..32), bottom holds padded rows 32..57 (input rows 31..55 + zero row).
# 2-row halo; conv group A (out rows 0..31) reads top, group B bottom.
TROWS = 34          # padded rows in top tile
BROWS = 26          # padded rows in bottom tile (incl. trailing zero row)
TXAF = 2112         # SLK + 34*58 + slack, %16 == 0
BXAF = 1584         # SLK + 26*58 + slack, %16 == 0
CH_ROWS = 8         # output rows per matmul chunk
CHF = CH_ROWS * PW  # matmul free size (464 <= 512)
NCH = 7             # chunks per image (56 rows)
N_TOT = float(32 * H * W)   # BN count over the full batch
BN_EPS = 1e-5
USE_FP8 = True      # fp8e4m3 DoubleRow matmuls (exact: values are -1/0/+1)
import os as _os
SIGN_MODE = _os.environ.get("K_SIGN", "act")    # act | split | pool
SQ_MODE = _os.environ.get("K_SQ", "act")        # act | dve | split

F32 = mybir.dt.float32
BF16 = mybir.dt.bfloat16
FP8 = mybir.dt.float8e4
XA_DT = FP8 if USE_FP8 else BF16

TAPS = [(kh, kw) for kh in range(3) for kw in range(3)]


def build_nc():
    nc = bacc.Bacc("TRN2", target_bir_lowering=False, debug=False,
                   num_devices=N_CORES)
    x_d = nc.dram_tensor("x", [B, C, H, W], BF16, kind="ExternalInput").ap()
    w_d = nc.dram_tensor("wsb", [128, 36 * 128], XA_DT, kind="ExternalInput").ap()
    p_d = nc.dram_tensor("params", [128, 6], F32, kind="ExternalInput").ap()
    o_d = nc.dram_tensor("out", [B, C, H, W], F32, kind="ExternalOutput").ap()

    with tile.TileContext(nc) as tc:
        with tc.tile_pool(name="wp", bufs=1) as wp, \
             tc.tile_pool(name="xap", bufs=B) as xap, \
             tc.tile_pool(name="yp", bufs=1) as yp, \
             tc.tile_pool(name="xsp", bufs=3) as xsp, \
             tc.tile_pool(name="stp", bufs=1) as stp, \
             tc.tile_pool(name="scrp", bufs=2) as scrp, \
             tc.tile_pool(name="psp", bufs=2, space="PSUM") as psp, \
             tc.tile_pool(name="drp", bufs=1, space="DRAM") as drp:

            # --- resident tensors (weight DMAs are emitted after image 0's
            # activation pieces: first matmul needs weights only ~4us in) ---
            wt = wp.tile([128, 36 * 128], XA_DT)
            # packed as [p, tap, ci, co, f]
            wv = wt[:, :].rearrange("p (t c o f) -> p t c o f", t=9, c=2, o=2)
            pt = wp.tile([128, 6], F32)

            xat, xab = [], []
            for b in range(B):
                t = xap.tile([128, 2, TXAF], XA_DT, name=f"xat{b}", tag="xat")
                u = xap.tile([128, 2, BXAF], XA_DT, name=f"xab{b}", tag="xab")
                for ci in range(2):
                    # zero the regions sign() won't overwrite: slack, pad
                    # rows, and the (right,left) pad column pairs.
                    nc.any.memset(t[:, ci, 0:SLK + PW], 0.0)
                    nc.any.memset(t[:, ci, SLK + TROWS * PW:TXAF], 0.0)
                    tp = t[:, ci, SLK + 57:SLK + 57 + (TROWS - 1) * PW]
                    nc.any.memset(
                        tp.rearrange("p (r c) -> p r c", c=PW)[:, :, 0:2], 0.0)
                    nc.any.memset(t[:, ci, SLK + 57 + (TROWS - 1) * PW:
                                    SLK + 57 + (TROWS - 1) * PW + 2], 0.0)
                    nc.any.memset(u[:, ci, 0:SLK], 0.0)
                    nc.any.memset(u[:, ci, SLK + 25 * PW:BXAF], 0.0)
                    up = u[:, ci, SLK - 1:SLK - 1 + 26 * PW]
                    nc.any.memset(
                        up.rearrange("p (r c) -> p r c", c=PW)[:, :, 0:2], 0.0)
                xat.append(t)
                xab.append(u)

            yt = [yp.tile([128, B, H * W], F32, name=f"yt{t}")
                  for t in range(2)]

            # 9 stat columns per cout tile; separate tiles per cout so
            # co=0's reduce doesn't dep-wait on co=1's conv (tile-granular deps)
            ssum = [stp.tile([128, 9], F32, name=f"ssum{t}") for t in range(2)]
            ssq = [stp.tile([128, 9], F32, name=f"ssq{t}") for t in range(2)]

            # --- load x and binarize into padded fp8 activations.
            # pieces: (T) input rows 0..32 -> top tile rows 1..33,
            #         (Bt) input rows 31..55 -> bottom tile rows 0..24.
            # For image 0, the ci=1 pieces run on DVE (sign ==
            # min(max(x*1e38,-1),1)) so ACT and DVE binarize in parallel
            # ahead of the first matmul group.
            def emit_piece(b, ci, x0, nr, dr, is_top, on_dve, k):
                xs = xsp.tile([128, TROWS, W], BF16, tag="xs",
                              name=f"xs_{b}_{ci}_{k}")
                nc.sync.dma_start(
                    out=xs[:, 0:nr, :],
                    in_=x_d[b, ci * 128:(ci + 1) * 128, x0:x0 + nr, :])
                tile_ = (xat if is_top else xab)[b]
                nrows = TROWS if is_top else BROWS
                interior = tile_[:, ci, SLK:SLK + nrows * PW].rearrange(
                    "p (h w) -> p h w", w=PW)
                dst8 = interior[:, dr:dr + nr, 1:W + 1]
                if on_dve:
                    sgt = scrp.tile([128, TROWS, W], F32, tag="sgt",
                                    name=f"sgt_{b}_{ci}_{k}")
                    nc.vector.tensor_scalar(
                        out=sgt[:, 0:nr, :], in0=xs[:, 0:nr, :],
                        scalar1=1e38, scalar2=-1.0,
                        op0=mybir.AluOpType.mult, op1=mybir.AluOpType.max)
                    nc.vector.tensor_scalar_min(dst8, sgt[:, 0:nr, :], 1.0)
                else:
                    nc.scalar.sign(out=dst8, in_=xs[:, 0:nr, :])

            PIECES = [(0, 33, 1, True), (31, 25, 0, False)]  # (x0, rows, dr, top)
            # image 0 first, top pieces split in half, ci1 on DVE
            for x0, nr, dr, is_top in PIECES:
                h1 = nr // 2
                for ci in range(2):
                    emit_piece(0, ci, x0, h1, dr, is_top, ci == 1, 0)
                    emit_piece(0, ci, x0 + h1, nr - h1, dr + h1, is_top,
                               ci == 1, 1)
            nc.sync.dma_start(out=wt[:, :], in_=w_d[:, :])
            nc.sync.dma_start(out=pt[:, :], in_=p_d[:, :])
            for t_ in range(2):
                nc.any.memset(ssum[t_][:, :], 0.0)
                nc.any.memset(ssq[t_][:, :], 0.0)
            for b in range(1, B):
                for ci in range(2):
                    for k, (x0, nr, dr, is_top) in enumerate(PIECES):
                        emit_piece(b, ci, x0, nr, dr, is_top, False, k)

            # --- per-cout-tile tail: reduce stats, all-reduce, coefficients,
            # epilogue + out-DMA. co=0's chain was already emitted inside the
            # conv loop (its stats close ~6us before co=1's), so its
            # AllReduce + epilogue + DMA overlap the rest of conv and AR #2.
            pv = pt[:, :].rearrange("p (t j) -> p t j", j=3)
            HALF = (H // 2) * W  # 28 rows

            def emit_tail(co):
                tot = stp.tile([128, 2], F32, name=f"tot{co}")
                nc.vector.tensor_reduce(
                    out=tot[:, 0:1], in_=ssum[co][:, :],
                    axis=mybir.AxisListType.X, op=mybir.AluOpType.add)
                nc.vector.tensor_reduce(
                    out=tot[:, 1:2], in_=ssq[co][:, :],
                    axis=mybir.AxisListType.X, op=mybir.AluOpType.add)
                cc_in = drp.tile([128, 2], F32, name=f"cc_in{co}")
                cc_out = drp.tile([128, 2], F32, addr_space="Shared",
                                  name=f"cc_out{co}")
                nc.sync.dma_start(out=cc_in[:, :], in_=tot[:, :])
                nc.gpsimd.collective_compute(
                    "AllReduce", mybir.AluOpType.add,
                    replica_groups=[list(range(N_CORES))],
                    ins=[cc_in[:, :].opt()], outs=[cc_out[:, :].opt()])
                allst = stp.tile([128, 2], F32, name=f"allst{co}")
                nc.sync.dma_start(out=allst[:, :], in_=cc_out[:, :])

                # a = gamma*scale*rsqrt(scale^2*var+eps), b = beta - a*mean
                sc, ga, be = (pv[:, co, 0:1], pv[:, co, 1:2], pv[:, co, 2:3])
                mean = stp.tile([128, 1], F32, name=f"mean{co}")
                var = stp.tile([128, 1], F32, name=f"var{co}")
                d = stp.tile([128, 1], F32, name=f"d{co}")
                inv = stp.tile([128, 1], F32, name=f"inv{co}")
                acoef = stp.tile([128, 1], F32, name=f"acoef{co}")
                bcoef = stp.tile([128, 1], F32, name=f"bcoef{co}")
                tmp = stp.tile([128, 1], F32, name=f"tmp{co}")
                nc.vector.tensor_scalar_mul(mean[:, :], allst[:, 0:1],
                                            1.0 / N_TOT)
                nc.vector.scalar_tensor_tensor(
                    out=var[:, :], in0=mean[:, :], scalar=-1.0,
                    in1=mean[:, :],
                    op0=mybir.AluOpType.mult, op1=mybir.AluOpType.mult)
                nc.vector.tensor_scalar(
                    out=tmp[:, :], in0=allst[:, 1:2], scalar1=1.0 / N_TOT,
                    scalar2=None, op0=mybir.AluOpType.mult)
                nc.vector.tensor_add(var[:, :], var[:, :], tmp[:, :])
                nc.vector.tensor_mul(tmp[:, :], sc, sc)
                nc.vector.tensor_mul(d[:, :], tmp[:, :], var[:, :])
                nc.vector.tensor_scalar_add(d[:, :], d[:, :], BN_EPS)
                nc.scalar.sqrt(d[:, :], d[:, :])
                nc.vector.reciprocal(inv[:, :], d[:, :])
                nc.vector.tensor_mul(tmp[:, :], ga, sc)
                nc.vector.tensor_mul(acoef[:, :], tmp[:, :], inv[:, :])
                nc.vector.tensor_mul(tmp[:, :], acoef[:, :], mean[:, :])
                nc.vector.tensor_sub(bcoef[:, :], be, tmp[:, :])

                # epilogue: out = relu(a*s + b) in place, ACT/DVE halves,
                # DMA out per half so the write streams immediately.
                for i, (b, h) in enumerate((b, h) for b in range(B)
                                           for h in range(2)):
                    yv = yt[co][:, b, h * HALF:(h + 1) * HALF]
                    if i % 2 == 0:
                        nc.scalar.activation(
                            out=yv, in_=yv,
                            func=mybir.ActivationFunctionType.Relu,
                            scale=acoef[:, 0:1], bias=bcoef[:, 0:1])
                    else:
                        nc.vector.tensor_scalar(
                            out=yv, in0=yv,
                            scalar1=acoef[:, 0:1], scalar2=bcoef[:, 0:1],
                            op0=mybir.AluOpType.mult, op1=mybir.AluOpType.add)
                        nc.vector.tensor_scalar_max(yv, yv, 0.0)
                    nc.sync.dma_start(
                        out=o_d[b, co * 128:(co + 1) * 128,
                                h * (H // 2):(h + 1) * (H // 2), :],
                        in_=yv.rearrange("p (h w) -> p h w", w=W))


            warm = stp.tile([128, 2], F32)
            warmed = [False]

            # --- conv: 9-tap shifted matmuls; multi-bank psum groups.
            # Block order interleaves cout tiles so co=0 finishes two blocks
            # before conv end: its AllReduce + epilogue + out-DMA overlap the
            # remaining co=1 conv.
            CONV_ORDER = [(b, co) for b in range(B) for co in range(2)]
            for b, co in CONV_ORDER:
                if True:
                    # the very last group is a single chunk so the final
                    # stats (copy+square+reduce) close quickly after conv.
                    last = (b == B - 1 and co == 1)
                    GROUPS = ([(0, 4), (4, 2), (6, 1)] if last
                              else [(0, 4), (4, 3)])
                    for gi, (c0, ng) in enumerate(GROUPS):
                        pst = psp.tile([128, 4, 512], F32, tag="ps",
                                       name=f"ps_{co}_{b}_{gi}")
                        atile = xat[b] if gi == 0 else xab[b]

                        def _base(j, toff, _c0=c0, _gi=gi):
                            ch = _c0 + j
                            row0 = (1 + CH_ROWS * ch) if _gi == 0 \
                                else (CH_ROWS * ch - 31)
                            return SLK + row0 * PW + toff

                        if USE_FP8:
                            # DoubleRow: one matmul contracts both cin halves
                            for tap, (kh, kw) in enumerate(TAPS):
                                lhsT = wv[:, tap, :, co, :]
                                toff = (kh - 1) * PW + (kw - 1)
                                for j in range(ng):
                                    base = _base(j, toff)
                                    nc.tensor.matmul(
                                        out=pst[:, j, 0:CHF], lhsT=lhsT,
                                        rhs=atile[:, :, base:base + CHF],
                                        perf_mode=mybir.MatmulPerfMode.DoubleRow,
                                        start=(tap == 0), stop=(tap == 8))
                        else:
                            idx = 0
                            for ci in range(2):
                                for tap, (kh, kw) in enumerate(TAPS):
                                    lhsT = wv[:, tap, ci, co, :]
                                    toff = (kh - 1) * PW + (kw - 1)
                                    for j in range(ng):
                                        base = _base(j, toff)
                                        nc.tensor.matmul(
                                            out=pst[:, j, 0:CHF], lhsT=lhsT,
                                            rhs=atile[:, ci, base:base + CHF],
                                            start=(idx == 0), stop=(idx == 17))
                                    idx += 1
                        # evacuate whole group: DVE copy+sum, GpSimd sq+sum
                        src = pst[:, 0:ng, 0:CHF].rearrange(
                            "p g (r c) -> p g r c", c=PW)[:, :, :, 1:W + 1]
                        y0 = c0 * CH_ROWS * W
                        dst = yt[co][:, b, y0:y0 + ng * CH_ROWS * W].rearrange(
                            "p (g r c) -> p g r c", r=CH_ROWS, c=W)
                        col = 8 if gi == 2 else b * 2 + gi
                        nc.vector.tensor_scalar(
                            out=dst, in0=src, scalar1=0.0, scalar2=None,
                            op0=mybir.AluOpType.add, op1=mybir.AluOpType.add,
                            accum_out=ssum[co][:, col:col + 1])
                        scr = scrp.tile([128, 4 * CH_ROWS * W], F32, tag="scr")
                        sv = scr[:, 0:ng * CH_ROWS * W]
                        use_act_sq = (SQ_MODE == "act"
                                      or (SQ_MODE == "split" and gi == 0))
                        if use_act_sq:
                            # read the SBUF copy (dst), not PSUM: the psum
                            # slot must be released by the DVE copy alone.
                            nc.scalar.activation(
                                out=sv.rearrange("p (g r c) -> p g r c",
                                                 r=CH_ROWS, c=W),
                                in_=dst,
                                func=mybir.ActivationFunctionType.Square,
                                accum_out=ssq[co][:, col:col + 1])
                        else:
                            nc.vector.scalar_tensor_tensor(
                                out=sv,
                                in0=dst.rearrange("p g r c -> p (g r c)"),
                                scalar=1.0,
                                in1=dst.rearrange("p g r c -> p (g r c)"),
                                op0=mybir.AluOpType.mult,
                                op1=mybir.AluOpType.mult,
                                accum_out=ssq[co][:, col:col + 1])
                        if not warmed[0]:
                            # warm the ACT function tables used post-AllReduce
                            # (Sqrt/Relu) mid-conv: anchored on the first stats
                            # column so they run when ACT has slack, not at
                            # t=0 and not on the coefficient critical path.
                            warmed[0] = True
                            nc.scalar.sqrt(warm[:, 0:1], ssq[co][:, col:col + 1])
                            nc.scalar.activation(
                                warm[:, 1:2], ssum[co][:, col:col + 1],
                                mybir.ActivationFunctionType.Relu)

                    if b == B - 1 and co == 0:
                        # co=0 stats are complete: launch its AllReduce +
                        # epilogue now so they overlap co=1's conv tail.
                        emit_tail(0)

            emit_tail(1)
    nc.compile()
    return nc


def prep_weights(weight, gamma, beta):
    """Host-side parameter prep (tiny): per-channel scale + packed sign weights."""
    w = np.asarray(weight, dtype=np.float32)
    scale = np.mean(np.abs(w), axis=(1, 2, 3))            # [256]
    sw = np.sign(w).astype(np.float32)                    # [O, I, 3, 3]
    # wsb[p, tap, ci, co, f] = sw[co*128+f, ci*128+p, kh, kw]
    swf = sw.reshape(256, 256, 9)                          # [O, I, tap]
    wsb = np.empty((128, 9, 2, 2, 128), dtype=np.float32)
    for tap in range(9):
        for ci in range(2):
            for co in range(2):
                blk = swf[co * 128:(co + 1) * 128, ci * 128:(ci + 1) * 128, tap]
                wsb[:, tap, ci, co, :] = blk.T              # [I128, O128]
    np_dt = mybir.dt.np(XA_DT)
    wsb = wsb.reshape(128, 36 * 128).astype(np_dt)

    params = np.empty((128, 6), dtype=np.float32)
    for t in range(2):
        params[:, t * 3 + 0] = scale[t * 128:(t + 1) * 128]
        params[:, t * 3 + 1] = np.asarray(gamma, np.float32)[t * 128:(t + 1) * 128]
        params[:, t * 3 + 2] = np.asarray(beta, np.float32)[t * 128:(t + 1) * 128]
    return wsb, params


_NC_CACHE = {}


def _get_nc():
    if "nc" not in _NC_CACHE:
        _NC_CACHE["nc"] = build_nc()
    return _NC_CACHE["nc"]


def kernel(x, weight, gamma, beta):
    # fp32 -> bf16 preserves sign exactly (same exponent range); the device
    # only consumes sign(x), so this halves the x transfer losslessly.
    x = np.asarray(x, dtype=np.float32).astype(ml_dtypes.bfloat16)
    wsb, params = prep_weights(weight, gamma, beta)
    nc = _get_nc()
    in_maps = []
    for c in range(N_CORES):
        in_maps.append({
            "x": np.ascontiguousarray(x[c * B:(c + 1) * B]),
            "wsb": wsb,
            "params": params,
        })
    res = run_bass_kernel_spmd(nc, in_maps, core_ids=list(range(N_CORES)))
    out = np.concatenate([res.results[c]["out"] for c in range(N_CORES)], axis=0)
    return out.astype(np.float32)
